# revision 1
# baseline (speedup 1.0000x reference)
"""DeepGraphSAGE on Trainium2, 8-core SPMD Bass kernel.

Strategy (self-contained; shapes hardcoded for the target problem):
  - Nodes are partitioned contiguously across 8 cores (6250 rows each);
    each core owns the edges whose *destination* lands in its partition.
  - Mean-aggregation is computed edge-parallel: sorted-by-dst edges are
    gathered 128 at a time with `dma_gather` (one HBM row per edge) and
    segment-summed into PSUM with indicator matmuls (indicator built
    on-device from per-edge slot ids with one `tensor_scalar` is_equal).
  - Layers 2..5 aggregate *transformed* features T = X @ Wl^T (computed
    locally, AllGathered in bf16), so the aggregation output is already
    the `agg @ Wl^T` term. The `x @ Wr^T + b` term is computed locally
    per layer ("h_wr") and added in.
  - GraphNorm statistics are raw per-(graph, channel) sums/sumsqs taken
    with membership matmuls on PE, AllReduced across cores (one 80 KB
    collective per norm layer), turned into per-(graph, channel) affine
    A/B coefficients, and applied via small membership matmuls + DVE.
"""

import math
import os

import numpy as np
import ml_dtypes

import concourse.bacc as bacc
import concourse.bass as bass
import concourse.tile as tile
from concourse import bass_utils, mybir

BF = ml_dtypes.bfloat16
F32 = mybir.dt.float32
BF16 = mybir.dt.bfloat16
I16 = mybir.dt.int16
AF = mybir.ActivationFunctionType
ALU = mybir.AluOpType


class CFG:
    N = 50000
    E = 800000
    F = 50
    H = 512
    O = 121
    G = 20
    NC = 8
    EPS = 1e-5
    GDT = os.environ.get("KGDT", "bf16")
    S0 = 32768  # int16 gather index limit chunk boundary
    WIN = 128


def _ceil(a, b):
    return -(-a // b)


# --------------------------------------------------------------------------
# Host-side preprocessing
# --------------------------------------------------------------------------

def _plan(cfg, x, edge_index, batch):
    N, E, G, NC = cfg.N, cfg.E, cfg.G, cfg.NC
    F2 = _ceil(cfg.F, 64) * 64
    O2 = _ceil(cfg.O, 128) * 128
    CT = cfg.H // 128
    R = N // NC
    W = _ceil(R, cfg.WIN)
    WP = W * cfg.WIN
    S0 = min(cfg.S0, N)

    src = np.asarray(edge_index[0], dtype=np.int64)
    dst = np.asarray(edge_index[1], dtype=np.int64)
    batch = np.asarray(batch, dtype=np.int64)
    x = np.asarray(x, dtype=np.float32)

    deg = np.bincount(dst, minlength=N).astype(np.float32)
    invd = 1.0 / np.maximum(deg, 1.0)
    sz = np.bincount(batch, minlength=G).astype(np.float32)
    inv_sz = 1.0 / np.maximum(sz, 1.0)

    # per-core edge grouping by (window, chunk), sorted by local dst
    per_core = []
    counts = np.zeros((NC, W, 2), dtype=np.int64)
    for c in range(NC):
        sel = (dst >= c * R) & (dst < (c + 1) * R)
        d = dst[sel] - c * R
        s = src[sel]
        w = d >> 7
        k = (s >= S0).astype(np.int64)
        order = np.lexsort((d, k, w))
        d, s, k, w = d[order], s[order], k[order], w[order]
        counts[c] = np.bincount(w * 2 + k, minlength=W * 2).reshape(W, 2)
        per_core.append((d, s))

    nblk = _ceil(np.max(counts, axis=0), cfg.WIN)  # [W, 2]
    nblkA, nblkB = nblk[:, 0], nblk[:, 1]
    BTOT = int(nblk.sum())
    LA = int(nblkA.sum()) * cfg.WIN
    LB = int(nblkB.sum()) * cfg.WIN

    data = []
    for c in range(NC):
        d, s = per_core[c]
        idxa = np.zeros(max(LA, 16), dtype=np.int16)
        idxb = np.zeros(max(LB, 16), dtype=np.int16)
        slots = np.full(BTOT * cfg.WIN, 255.0, dtype=np.float32)
        pa = pb = pg = 0  # positions into idxa / idxb / slots
        pos = 0
        for wv in range(W):
            for kk in (0, 1):
                cnt = int(counts[c, wv, kk])
                B = int(nblk[wv, kk])
                seg_s = s[pos:pos + cnt] - kk * S0
                seg_d = d[pos:pos + cnt] & 127
                pos += cnt
                if kk == 0:
                    idxa[pa:pa + cnt] = seg_s
                    pa += B * cfg.WIN
                else:
                    idxb[pb:pb + cnt] = seg_s
                    pb += B * cfg.WIN
                slots[pg:pg + cnt] = seg_d
                pg += B * cfg.WIN
        assert pos == len(d)

        b_own = batch[c * R:(c + 1) * R]
        memb = np.zeros((WP, G), dtype=np.float32)
        memb[np.arange(R), b_own] = 1.0
        x_own = x[c * R:(c + 1) * R]
        xT = np.zeros((F2, WP), dtype=np.float32)
        xT[:cfg.F, :R] = x_own.T
        invd_own = np.ones(WP, dtype=np.float32)
        invd_own[:R] = invd[c * R:(c + 1) * R]

        data.append(dict(
            idxA=np.tile(idxa.reshape(-1, 16).T, (8, 1)).copy(),
            idxB=np.tile(idxb.reshape(-1, 16).T, (8, 1)).copy(),
            slots=slots.reshape(BTOT, cfg.WIN).T.copy(),
            invd=invd_own.reshape(W, cfg.WIN).T.copy(),
            memb=memb.reshape(W, cfg.WIN, G).transpose(1, 0, 2).reshape(cfg.WIN, W * G).copy(),
            membT=memb.T.copy(),
            xT=xT,
        ))

    x_pad = np.zeros((N, F2), dtype=np.float32)
    x_pad[:, :cfg.F] = x

    inv_szt = np.tile(inv_sz, (cfg.WIN, CT)).astype(np.float32)  # [WIN, CT*G]

    struct = dict(
        F2=F2, O2=O2, CT=CT, R=R, W=W, WP=WP, S0=S0,
        nblkA=[int(v) for v in nblkA], nblkB=[int(v) for v in nblkB],
        LA=LA, LB=LB, BTOT=BTOT,
    )
    shared = dict(
        x_pad=x_pad,
        inv_szt=inv_szt,
        iota=np.tile(np.arange(128, dtype=np.float32), (128, 1)).copy(),
    )
    return struct, shared, data


def _prep_weights(cfg, st, inp):
    """Host-side packing of the (replicated) weight/norm tensors."""
    H, O, G = cfg.H, cfg.O, cfg.G
    F2, O2, CT = st["F2"], st["O2"], st["CT"]

    def ktiled(wT, fo):  # [H, fo] -> [128, CT*fo] (k-tile major SBUF layout)
        return wT.reshape(CT, 128, fo).transpose(1, 0, 2).reshape(128, CT * fo).copy()

    out = {}
    w1lT = np.zeros((F2, H), np.float32)
    w1lT[:cfg.F] = np.asarray(inp["W1l"], np.float32).T
    w1rT = np.zeros((F2, H), np.float32)
    w1rT[:cfg.F] = np.asarray(inp["W1r"], np.float32).T
    out["w1l"] = w1lT
    out["w1r"] = w1rT
    out["b1"] = np.asarray(inp["b1"], np.float32).reshape(1, H)
    for l in (2, 3, 4):
        out[f"w{l}l"] = ktiled(np.asarray(inp[f"W{l}l"], np.float32).T, H)
        out[f"w{l}r"] = ktiled(np.asarray(inp[f"W{l}r"], np.float32).T, H)
        out[f"b{l}"] = np.asarray(inp[f"b{l}"], np.float32).reshape(1, H)
    w5lT = np.zeros((H, O2), np.float32)
    w5lT[:, :O] = np.asarray(inp["W5l"], np.float32).T
    out["w5l"] = ktiled(w5lT, O2)
    out["w5r"] = ktiled(np.asarray(inp["W5r"], np.float32).T, O)
    out["b5"] = np.asarray(inp["b5"], np.float32).reshape(1, O)

    for l in (1, 2, 3, 4):
        a = np.asarray(inp[f"a{l}"], np.float32)
        g = np.asarray(inp[f"g{l}"], np.float32)
        bn = np.asarray(inp[f"bn{l}"], np.float32)
        acoef = 2.0 * a - a * a
        # [128, 4*CT]; col p*CT+ct; params p: 0 alpha, 1 acoef, 2 w, 3 bn
        m = np.zeros((128, 4 * CT), np.float32)
        for ct in range(CT):
            cs = slice(ct * 128, (ct + 1) * 128)
            m[:, 0 * CT + ct] = a[cs]
            m[:, 1 * CT + ct] = acoef[cs]
            m[:, 2 * CT + ct] = g[cs]
            m[:, 3 * CT + ct] = bn[cs]
        out[f"nrm{l}"] = m
    return out


# --------------------------------------------------------------------------
# Device program
# --------------------------------------------------------------------------

def _build(cfg, st):
    N, H, O, G, NC = cfg.N, cfg.H, cfg.O, cfg.G, cfg.NC
    F2, O2, CT = st["F2"], st["O2"], st["CT"]
    R, W, WP, S0 = st["R"], st["W"], st["WP"], st["S0"]
    nblkA, nblkB = st["nblkA"], st["nblkB"]
    LA, LB, BTOT = st["LA"], st["LB"], st["BTOT"]
    CTG = CT * G
    RG = [list(range(NC))]
    TDT = BF16 if cfg.GDT == "bf16" else F32
    ABL = os.environ.get("KABL", "none")
    gmaxA = max(nblkA) if nblkA else 1
    gmaxB = max(nblkB) if nblkB else 1

    nc = bacc.Bacc(
        "TRN2",
        target_bir_lowering=False,
        debug=False,
        num_devices=NC,
        enable_asserts=False,
    )

    # ---- I/O ----
    din = {}
    def inp(name, shape, dt):
        din[name] = nc.dram_tensor(name, shape, dt, kind="ExternalInput")
        return din[name]

    x_pad = inp("x_pad", [N, F2], F32)
    xT = inp("xT", [F2, WP], F32)
    idxA = inp("idxA", [128, max(LA, 16) // 16], I16)
    idxB = inp("idxB", [128, max(LB, 16) // 16], I16)
    slots = inp("slots", [128, BTOT], F32)
    invd = inp("invd", [128, W], F32)
    memb = inp("memb", [128, W * G], F32)
    membT = inp("membT", [G, WP], F32)
    inv_szt = inp("inv_szt", [128, CTG], F32)
    iota = inp("iota", [128, 128], F32)
    for l in (1, 2, 3, 4):
        inp(f"nrm{l}", [128, 4 * CT], F32)
    inp("w1l", [F2, H], F32); inp("w1r", [F2, H], F32); inp("b1", [1, H], F32)
    for l in (2, 3, 4):
        inp(f"w{l}l", [128, CT * H], F32)
        inp(f"w{l}r", [128, CT * H], F32)
        inp(f"b{l}", [1, H], F32)
    inp("w5l", [128, CT * O2], F32); inp("w5r", [128, CT * O], F32)
    inp("b5", [1, O], F32)

    out_d = nc.dram_tensor("out", [R, O], F32, kind="ExternalOutput")

    import contextlib
    _ctx = contextlib.ExitStack()
    with tile.TileContext(nc) as tc:
        cpool = _ctx.enter_context(tc.tile_pool(name="cpool", bufs=1))
        wpool = _ctx.enter_context(tc.tile_pool(name="wpool", bufs=2))
        gpool = _ctx.enter_context(tc.tile_pool(name="gpool", bufs=int(os.environ.get("KGBUFS", "2"))))
        ipool = _ctx.enter_context(tc.tile_pool(name="ipool", bufs=3))
        work = _ctx.enter_context(tc.tile_pool(name="work", bufs=2))
        pseg = _ctx.enter_context(tc.tile_pool(name="pseg", bufs=2, space="PSUM"))
        pstat = _ctx.enter_context(tc.tile_pool(name="pstat", bufs=2, space="PSUM"))
        pwork = _ctx.enter_context(tc.tile_pool(name="pwork", bufs=4, space="PSUM"))
        dram = _ctx.enter_context(tc.tile_pool(name="dram", bufs=1, space="DRAM"))

        # ---- internal DRAM ----
        h_pre = dram.tile([WP, H], F32, tag="h_pre", name="h_pre")
        h_wr = {l: dram.tile([WP, H if l < 5 else O], F32, tag=f"h_wr{l}", name=f"h_wr{l}")
                for l in (2, 3, 4, 5)}
        t_in = {l: dram.tile([R, H if l < 5 else O2], TDT, tag=f"t_in{l}", name=f"t_in{l}")
                for l in (2, 3, 4, 5)}
        shared_as = "Shared" if NC > 4 else "Local"
        t_full = {l: dram.tile([N, H if l < 5 else O2], TDT, tag=f"t_full{l}",
                               name=f"t_full{l}", addr_space=shared_as)
                  for l in (2, 3, 4, 5)}
        ar_in = {l: dram.tile([128, 2 * CTG], F32, tag=f"ar_in{l}", name=f"ar_in{l}")
                 for l in (1, 2, 3, 4)}
        ar_out = {l: dram.tile([128, 2 * CTG], F32, tag=f"ar_out{l}",
                               name=f"ar_out{l}", addr_space=shared_as)
                  for l in (1, 2, 3, 4)}

        # ---- resident constants ----
        def cload(name):
            t = din[name]
            tl = cpool.tile(list(t.shape), t.dtype, name=f"{name}_sb")
            nc.sync.dma_start(tl[:], t.ap())
            return tl

        idxA_sb = cload("idxA")
        idxB_sb = cload("idxB")
        slots_sb = cload("slots")
        invd_sb = cload("invd")
        memb_sb = cload("memb")
        inv_szt_sb = cload("inv_szt")
        iota_sb = cload("iota")
        nrm_sb = {l: cload(f"nrm{l}") for l in (1, 2, 3, 4)}
        w1l_sb = cload("w1l"); w1r_sb = cload("w1r"); b1_sb = cload("b1")

        ident = cpool.tile([128, 128], F32, name="ident")
        from concourse.masks import make_identity
        make_identity(nc, ident[:])
        ones1 = cpool.tile([1, 128], F32, name="ones1")
        nc.vector.memset(ones1[:], 1.0)
        eps_col = cpool.tile([128, 1], F32, name="eps_col")
        nc.vector.memset(eps_col[:], cfg.EPS)

        # block offset bookkeeping (same for every layer)
        eA0 = np.concatenate([[0], np.cumsum(nblkA)]).astype(int)   # in blocks
        eB0 = np.concatenate([[0], np.cumsum(nblkB)]).astype(int)
        sc0 = np.concatenate([[0], np.cumsum(np.asarray(nblkA) + np.asarray(nblkB))]).astype(int)

        def phase_a(l):
            """gather + segment matmul + local term; h_pre/stats (l<=4) or out (l==5)."""
            if l == 1:
                src, elem, sdt = x_pad, F2, F32
            elif l < 5:
                src, elem, sdt = t_full[l], H, TDT
            else:
                src, elem, sdt = t_full[5], O2, TDT
            segw = elem if l == 1 else (H if l < 5 else O2)

            if l <= 4:
                stats = pstat.tile([128, 2 * CTG], F32, tag="stats", name=f"stats{l}")

            nbmax = max(nblkA[w] + nblkB[w] for w in range(W))
            for w in range(W):
                bA, bB = nblkA[w], nblkB[w]
                nb = bA + bB
                gA = gB = None
                GMAXBLK = 8  # cap descriptors per dma_gather call
                if bA:
                    gA = gpool.tile([128, gmaxA, elem], sdt, tag="gA", name=f"gA_{l}_{w}")
                    for o in [] if ABL == "nogather" else range(0, bA, GMAXBLK):
                        n = min(GMAXBLK, bA - o)
                        nc.gpsimd.dma_gather(
                            out_ap=gA[:, o:o + n, :],
                            in_ap=src[:S0, :] if S0 < N else src[:, :],
                            idxs_ap=idxA_sb[:, (eA0[w] + o) * 8: (eA0[w] + o + n) * 8],
                            num_idxs=n * 128,
                            num_idxs_reg=n * 128,
                            elem_size=elem,
                        )
                if bB:
                    gB = gpool.tile([128, gmaxB, elem], sdt, tag="gB", name=f"gB_{l}_{w}")
                    for o in [] if ABL == "nogather" else range(0, bB, GMAXBLK):
                        n = min(GMAXBLK, bB - o)
                        nc.gpsimd.dma_gather(
                            out_ap=gB[:, o:o + n, :],
                            in_ap=src[S0:, :],
                            idxs_ap=idxB_sb[:, (eB0[w] + o) * 8: (eB0[w] + o + n) * 8],
                            num_idxs=n * 128,
                            num_idxs_reg=n * 128,
                            elem_size=elem,
                        )
                seg = pseg.tile([128, segw], F32, tag="seg", name=f"seg_{l}_{w}")
                if nb == 0:
                    nc.vector.memset(seg[:], 0.0)
                else:
                    # whole-window indicator build: ind[e, b, s] = (slot[e, b] == s)
                    ind = ipool.tile([128, nbmax, 128], sdt, tag="ind",
                                     name=f"ind_{l}_{w}")
                    sl = slots_sb[:, sc0[w]: sc0[w] + nb]
                    slots_b = bass.AP(tensor=sl.tensor, offset=sl.offset,
                                      ap=[list(sl.ap[0]), list(sl.ap[1]), [0, 128]])
                    iota_b = bass.AP(tensor=iota_sb.tensor, offset=iota_sb.offset,
                                     ap=[list(iota_sb.ap[0]), [0, nb],
                                         list(iota_sb.ap[1])])
                    nc.vector.tensor_tensor(out=ind[:, :nb, :], in0=iota_b,
                                            in1=slots_b, op=ALU.is_equal)
                bi = 0
                for gt, bcnt in ((gA, bA), (gB, bB)):
                    for b in range(bcnt):
                        if ABL == "noseg" and bi > 0:
                            bi += 1
                            continue
                        nc.tensor.matmul(
                            out=seg[:], lhsT=ind[:, bi, :], rhs=gt[:, b, :],
                            start=(bi == 0), stop=(bi == nb - 1 or ABL == "noseg"))
                        bi += 1

                r0 = w * 128
                if l == 1:
                    aggs = work.tile([128, F2], F32, tag="aggs", name=f"aggs_{w}")
                    nc.vector.tensor_scalar(out=aggs[:], in0=seg[:],
                                            scalar1=invd_sb[:, w:w + 1],
                                            scalar2=None, op0=ALU.mult)
                    tp = pwork.tile([F2, 128], F32, tag="wk", name=f"atp_{w}")
                    nc.tensor.transpose(tp[:], aggs[:], ident[:])
                    aT = work.tile([F2, 128], F32, tag="aT", name=f"aT_{w}")
                    nc.vector.tensor_copy(aT[:], tp[:])
                    xtt = work.tile([F2, 128], F32, tag="xtt", name=f"xtt_{w}")
                    nc.sync.dma_start(xtt[:], xT[:, r0:r0 + 128])
                    hp = pwork.tile([128, H], F32, tag="wk", name=f"hp_{w}")
                    nc.tensor.matmul(out=hp[:], lhsT=aT[:], rhs=w1l_sb[:],
                                     start=True, stop=False)
                    nc.tensor.matmul(out=hp[:], lhsT=xtt[:],
                                     rhs=w1r_sb[:], start=False, stop=False)
                    nc.tensor.matmul(out=hp[:], lhsT=ones1[:], rhs=b1_sb[:],
                                     start=False, stop=True)
                    h2 = work.tile([128, H], F32, tag="h2", name=f"h2_{l}_{w}")
                    nc.vector.tensor_copy(h2[:], hp[:])
                elif l < 5:
                    h1t = work.tile([128, H], F32, tag="h1t", name=f"h1t_{l}_{w}")
                    nc.vector.tensor_scalar(out=h1t[:], in0=seg[:],
                                            scalar1=invd_sb[:, w:w + 1],
                                            scalar2=None, op0=ALU.mult)
                    hw = work.tile([128, H], F32, tag="hw", name=f"hw_{l}_{w}")
                    nc.sync.dma_start(hw[:], h_wr[l][r0:r0 + 128, :])
                    h2 = work.tile([128, H], F32, tag="h2", name=f"h2_{l}_{w}")
                    nc.vector.tensor_tensor(out=h2[:], in0=h1t[:], in1=hw[:],
                                            op=ALU.add)
                else:
                    o1 = work.tile([128, O], F32, tag="o1", name=f"o1_{w}")
                    nc.vector.tensor_scalar(out=o1[:], in0=seg[:, :O],
                                            scalar1=invd_sb[:, w:w + 1],
                                            scalar2=None, op0=ALU.mult)
                    hw5 = work.tile([128, O], F32, tag="hw5", name=f"hw5_{w}")
                    nc.sync.dma_start(hw5[:], h_wr[5][r0:r0 + 128, :])
                    o2 = work.tile([128, O], F32, tag="o2", name=f"o2_{w}")
                    nc.vector.tensor_tensor(out=o2[:], in0=o1[:], in1=hw5[:],
                                            op=ALU.add)
                    rows = min(128, R - r0)
                    nc.sync.dma_start(out_d[r0:r0 + rows, :], o2[:rows, :])
                    continue

                # stats + h_pre store (l <= 4)
                hsq = work.tile([128, H], F32, tag="hsq", name=f"hsq_{l}_{w}")
                nc.vector.tensor_tensor(out=hsq[:], in0=h2[:], in1=h2[:],
                                        op=ALU.mult)
                for ct in range(CT):
                    nc.tensor.matmul(
                        out=stats[:, ct * G:(ct + 1) * G],
                        lhsT=h2[:, ct * 128:(ct + 1) * 128],
                        rhs=memb_sb[:, w * G:(w + 1) * G],
                        start=(w == 0 and ct == 0),
                        stop=False)
                    nc.tensor.matmul(
                        out=stats[:, CTG + ct * G: CTG + (ct + 1) * G],
                        lhsT=hsq[:, ct * 128:(ct + 1) * 128],
                        rhs=memb_sb[:, w * G:(w + 1) * G],
                        start=False,
                        stop=(w == W - 1 and ct == CT - 1))
                nc.sync.dma_start(h_pre[r0:r0 + 128, :], h2[:])
            if l <= 4:
                return stats
            return None

        def norm_coeffs(l, stats):
            """AllReduce raw moments; produce A_rhs/B_rhs [G, H] for phase C."""
            stsb = work.tile([128, 2 * CTG], F32, tag="stsb", name=f"stsb{l}")
            nc.vector.tensor_copy(stsb[:], stats[:])
            nc.sync.dma_start(ar_in[l][:, :], stsb[:])
            if ABL != "nocc":
                nc.gpsimd.collective_compute(
                    "AllReduce", ALU.add, replica_groups=RG,
                    ins=[ar_in[l].opt()], outs=[ar_out[l].opt()])
            stg = work.tile([128, 2 * CTG], F32, tag="stg", name=f"stg{l}")
            nc.sync.dma_start(stg[:], ar_out[l][:, :])

            nrm = nrm_sb[l]
            mean = work.tile([128, CTG], F32, tag="mean", name=f"mean{l}")
            nc.vector.tensor_tensor(out=mean[:], in0=stg[:, :CTG],
                                    in1=inv_szt_sb[:], op=ALU.mult)
            e2 = work.tile([128, CTG], F32, tag="e2", name=f"e2{l}")
            nc.vector.tensor_tensor(out=e2[:], in0=stg[:, CTG:],
                                    in1=inv_szt_sb[:], op=ALU.mult)
            msq = work.tile([128, CTG], F32, tag="msq", name=f"msq{l}")
            nc.vector.tensor_tensor(out=msq[:], in0=mean[:], in1=mean[:],
                                    op=ALU.mult)
            msq2 = work.tile([128, CTG], F32, tag="msq2", name=f"msq2{l}")
            for ct in range(CT):
                sl = slice(ct * G, (ct + 1) * G)
                nc.vector.tensor_scalar(out=msq2[:, sl], in0=msq[:, sl],
                                        scalar1=nrm[:, 1 * CT + ct: 1 * CT + ct + 1],
                                        scalar2=None, op0=ALU.mult)
            var = work.tile([128, CTG], F32, tag="var", name=f"var{l}")
            nc.vector.tensor_tensor(out=var[:], in0=e2[:], in1=msq2[:],
                                    op=ALU.subtract)
            sd = work.tile([128, CTG], F32, tag="sd", name=f"sd{l}")
            nc.scalar.activation(out=sd[:], in_=var[:], func=AF.Sqrt,
                                 bias=eps_col[:])
            rstd = work.tile([128, CTG], F32, tag="rstd", name=f"rstd{l}")
            nc.vector.reciprocal(rstd[:], sd[:])
            At = work.tile([128, CTG], F32, tag="At", name=f"At{l}")
            for ct in range(CT):
                sl = slice(ct * G, (ct + 1) * G)
                nc.vector.tensor_scalar(out=At[:, sl], in0=rstd[:, sl],
                                        scalar1=nrm[:, 2 * CT + ct: 2 * CT + ct + 1],
                                        scalar2=None, op0=ALU.mult)
            mA = work.tile([128, CTG], F32, tag="mA", name=f"mA{l}")
            nc.vector.tensor_tensor(out=mA[:], in0=mean[:], in1=At[:],
                                    op=ALU.mult)
            Bt = work.tile([128, CTG], F32, tag="Bt", name=f"Bt{l}")
            for ct in range(CT):
                sl = slice(ct * G, (ct + 1) * G)
                nc.vector.tensor_scalar(out=Bt[:, sl], in0=mA[:, sl],
                                        scalar1=nrm[:, 0 * CT + ct: 0 * CT + ct + 1],
                                        scalar2=nrm[:, 3 * CT + ct: 3 * CT + ct + 1],
                                        op0=ALU.mult, op1=ALU.subtract)
            A_rhs = work.tile([G, H], F32, tag="A_rhs", name=f"A_rhs{l}")
            B_rhs = work.tile([G, H], F32, tag="B_rhs", name=f"B_rhs{l}")
            for ct in range(CT):
                tpa = pwork.tile([G, 128], F32, tag="wk", name=f"tpa{l}_{ct}")
                nc.tensor.transpose(tpa[:], At[:, ct * G:(ct + 1) * G], ident[:])
                nc.vector.tensor_copy(A_rhs[:, ct * 128:(ct + 1) * 128], tpa[:])
                tpb = pwork.tile([G, 128], F32, tag="wk", name=f"tpb{l}_{ct}")
                nc.tensor.transpose(tpb[:], Bt[:, ct * G:(ct + 1) * G], ident[:])
                nc.vector.tensor_copy(B_rhs[:, ct * 128:(ct + 1) * 128], tpb[:])
            return A_rhs, B_rhs

        def phase_c(l, A_rhs, B_rhs):
            """normalize+relu h_pre, emit t_in(l+1) (bf16) and h_wr(l+1); AllGather."""
            lw = l + 1
            TW = H if lw < 5 else O2
            WW = H if lw < 5 else O
            wl = wpool.tile([128, CT * TW], F32, tag="wl", name=f"wl{lw}")
            nc.sync.dma_start(wl[:], din[f"w{lw}l"].ap())
            wr = wpool.tile([128, CT * WW], F32, tag="wr", name=f"wr{lw}")
            nc.sync.dma_start(wr[:], din[f"w{lw}r"].ap())
            br = wpool.tile([1, WW], F32, tag="br", name=f"br{lw}")
            nc.sync.dma_start(br[:], din[f"b{lw}"].ap())

            for mt in range(W):
                r0 = mt * 128
                hc = work.tile([128, H], F32, tag="hc", name=f"hc{l}_{mt}")
                nc.sync.dma_start(hc[:], h_pre[r0:r0 + 128, :])
                mTt = work.tile([G, 128], F32, tag="mTt", name=f"mTt{l}_{mt}")
                nc.sync.dma_start(mTt[:], membT[:, r0:r0 + 128])
                pa = pwork.tile([128, H], F32, tag="wk", name=f"pa{l}_{mt}")
                nc.tensor.matmul(out=pa[:], lhsT=mTt[:],
                                 rhs=A_rhs[:], start=True, stop=True)
                pb = pwork.tile([128, H], F32, tag="wk", name=f"pb{l}_{mt}")
                nc.tensor.matmul(out=pb[:], lhsT=mTt[:],
                                 rhs=B_rhs[:], start=True, stop=True)
                t0 = work.tile([128, H], F32, tag="t0", name=f"t0{l}_{mt}")
                nc.vector.tensor_tensor(out=t0[:], in0=hc[:], in1=pa[:],
                                        op=ALU.mult)
                t1 = work.tile([128, H], F32, tag="t1", name=f"t1{l}_{mt}")
                nc.vector.tensor_tensor(out=t1[:], in0=t0[:], in1=pb[:],
                                        op=ALU.subtract)
                xn = work.tile([128, H], F32, tag="xn", name=f"xn{l}_{mt}")
                nc.scalar.activation(out=xn[:], in_=t1[:], func=AF.Relu)
                xts = work.tile([128, CT, 128], F32, tag="xts", name=f"xts{l}_{mt}")
                for kt in range(CT):
                    tp = pwork.tile([128, 128], F32, tag="wk", name=f"tp{l}_{mt}_{kt}")
                    nc.tensor.transpose(tp[:], xn[:, kt * 128:(kt + 1) * 128], ident[:])
                    nc.vector.tensor_copy(xts[:, kt, :], tp[:])
                pt = pwork.tile([128, TW], F32, tag="wk", name=f"pt{l}_{mt}")
                for kt in range(CT):
                    nc.tensor.matmul(out=pt[:], lhsT=xts[:, kt, :],
                                     rhs=wl[:, kt * TW:(kt + 1) * TW],
                                     start=(kt == 0), stop=(kt == CT - 1))
                tsb = work.tile([128, TW], TDT, tag="tsb", name=f"tsb{l}_{mt}")
                nc.vector.tensor_copy(tsb[:], pt[:])
                rows = min(128, R - r0)
                nc.sync.dma_start(t_in[lw][r0:r0 + rows, :], tsb[:rows, :])
                pw = pwork.tile([128, WW], F32, tag="wk", name=f"pw{l}_{mt}")
                for kt in range(CT):
                    nc.tensor.matmul(out=pw[:], lhsT=xts[:, kt, :],
                                     rhs=wr[:, kt * WW:(kt + 1) * WW],
                                     start=(kt == 0), stop=False)
                nc.tensor.matmul(out=pw[:], lhsT=ones1[:], rhs=br[:],
                                 start=False, stop=True)
                wsb = work.tile([128, WW], F32, tag="wsb", name=f"wsb{l}_{mt}")
                nc.vector.tensor_copy(wsb[:], pw[:])
                nc.sync.dma_start(h_wr[lw][r0:r0 + 128, :], wsb[:])

            if ABL != "nocc":
                nc.gpsimd.collective_compute(
                    "AllGather", ALU.bypass, replica_groups=RG,
                    ins=[t_in[lw].opt()], outs=[t_full[lw].opt()])

        for l in (1, 2, 3, 4):
            stats = phase_a(l)
            A_rhs, B_rhs = norm_coeffs(l, stats)
            phase_c(l, A_rhs, B_rhs)
        phase_a(5)

        _ctx.close()

    nc.compile()
    return nc


# --------------------------------------------------------------------------
# Entry point
# --------------------------------------------------------------------------

def _run(cfg, inputs, trace=False):
    st, shared, data = _plan(cfg, inputs["x"], inputs["edge_index"], inputs["batch"])
    wts = _prep_weights(cfg, st, inputs)
    nc = _build(cfg, st)
    in_maps = []
    for c in range(cfg.NC):
        m = dict(shared)
        m.update(wts)
        m.update(data[c])
        in_maps.append(m)
    res = bass_utils.run_bass_kernel_spmd(
        nc, in_maps, core_ids=list(range(cfg.NC)), trace=trace)
    out = np.concatenate([res.results[c]["out"] for c in range(cfg.NC)], axis=0)
    return out.astype(np.float32), res


def kernel(**inputs) -> np.ndarray:
    out, _ = _run(CFG, inputs)
    return out



# revision 8
# speedup vs baseline: 1.5009x; 1.5009x over previous
"""DeepGraphSAGE on Trainium2, 8-core SPMD Bass kernel (v2).

Strategy (self-contained; shapes hardcoded for the target problem):
  - Nodes are partitioned contiguously across 8 cores (6250 rows each);
    each core owns the edges whose *destination* lands in its partition.
  - Mean-aggregation is computed edge-parallel: sorted-by-dst edges are
    gathered 128 at a time with `dma_gather` (one HBM row per edge) and
    segment-summed into PSUM with indicator matmuls (indicator built
    on-device from per-edge slot ids with one `tensor_tensor` is_equal).
  - Layers 2..5 aggregate *transformed* features T = X @ Wl^T (computed
    locally, AllGathered in fp8/bf16), so the aggregation output is
    already the `agg @ Wl^T` term. The `x @ Wr^T + b` term ("hwr") is
    recomputed per window in the NEXT layer's phase_a from the
    SBUF-resident transposed activations (xts_all), so it costs no DRAM
    round trip and its PE work overlaps the AllGather + gather DMAs.
  - GraphNorm statistics are raw per-(graph, channel) sums/sumsqs taken
    with membership matmuls on PE, AllReduced across cores (one 80 KB
    collective per norm layer), turned into per-(graph, channel) affine
    A/B coefficients, and applied via small membership matmuls + DVE.
  - All PE matmuls run in bf16/fp8 (1 cyc/row vs fp32's 4); gathered
    payloads for the H=512 layers are fp8_e4m3 (512 B descriptors, full
    DMA-engine rate).
"""

import math
import os

import numpy as np
import ml_dtypes

import concourse.bacc as bacc
import concourse.bass as bass
import concourse.tile as tile
from concourse import bass_utils, mybir

BF = ml_dtypes.bfloat16
F32 = mybir.dt.float32
BF16 = mybir.dt.bfloat16
FP8 = mybir.dt.float8e4
I16 = mybir.dt.int16
AF = mybir.ActivationFunctionType
ALU = mybir.AluOpType


class CFG:
    N = 50000
    E = 800000
    F = 50
    H = 512
    O = 121
    G = 20
    NC = 8
    EPS = 1e-5
    GDT = os.environ.get("KGDT", "fp8e4")  # t_full dtype for layers 2-4
    S0 = 32768  # int16 gather index limit chunk boundary
    WIN = 128


def _ceil(a, b):
    return -(-a // b)


# --------------------------------------------------------------------------
# Host-side preprocessing
# --------------------------------------------------------------------------

def _plan(cfg, x, edge_index, batch):
    N, E, G, NC = cfg.N, cfg.E, cfg.G, cfg.NC
    F2 = _ceil(cfg.F, 128) * 128
    O2 = _ceil(cfg.O, 128) * 128
    CT = cfg.H // 128
    R = N // NC
    W = _ceil(R, cfg.WIN)
    WP = W * cfg.WIN
    S0 = min(cfg.S0, N)

    src = np.asarray(edge_index[0], dtype=np.int64)
    dst = np.asarray(edge_index[1], dtype=np.int64)
    batch = np.asarray(batch, dtype=np.int64)
    x = np.asarray(x, dtype=np.float32)

    deg = np.bincount(dst, minlength=N).astype(np.float32)
    invd = 1.0 / np.maximum(deg, 1.0)
    sz = np.bincount(batch, minlength=G).astype(np.float32)
    inv_sz = 1.0 / np.maximum(sz, 1.0)

    # per-core edge grouping by (window, chunk), sorted by local dst
    per_core = []
    counts = np.zeros((NC, W, 2), dtype=np.int64)
    for c in range(NC):
        sel = (dst >= c * R) & (dst < (c + 1) * R)
        d = dst[sel] - c * R
        s = src[sel]
        w = d >> 7
        k = (s >= S0).astype(np.int64)
        order = np.lexsort((d, k, w))
        d, s, k, w = d[order], s[order], k[order], w[order]
        counts[c] = np.bincount(w * 2 + k, minlength=W * 2).reshape(W, 2)
        per_core.append((d, s))

    nblk = _ceil(np.max(counts, axis=0), cfg.WIN)  # [W, 2]
    nblkA, nblkB = nblk[:, 0], nblk[:, 1]
    BTOT = int(nblk.sum())
    LA = int(nblkA.sum()) * cfg.WIN
    LB = int(nblkB.sum()) * cfg.WIN

    data = []
    for c in range(NC):
        d, s = per_core[c]
        idxa = np.zeros(max(LA, 16), dtype=np.int16)
        idxb = np.zeros(max(LB, 16), dtype=np.int16)
        slots = np.full(BTOT * cfg.WIN, 255.0, dtype=np.float32)
        pa = pb = pg = 0  # positions into idxa / idxb / slots
        pos = 0
        for wv in range(W):
            for kk in (0, 1):
                cnt = int(counts[c, wv, kk])
                B = int(nblk[wv, kk])
                seg_s = s[pos:pos + cnt] - kk * S0
                seg_d = d[pos:pos + cnt] & 127
                pos += cnt
                if kk == 0:
                    idxa[pa:pa + cnt] = seg_s
                    pa += B * cfg.WIN
                else:
                    idxb[pb:pb + cnt] = seg_s
                    pb += B * cfg.WIN
                slots[pg:pg + cnt] = seg_d
                pg += B * cfg.WIN
        assert pos == len(d)

        b_own = batch[c * R:(c + 1) * R]
        memb = np.zeros((WP, G), dtype=np.float32)
        memb[np.arange(R), b_own] = 1.0
        x_own = x[c * R:(c + 1) * R]
        xT = np.zeros((F2, WP), dtype=BF)
        xT[:cfg.F, :R] = x_own.T.astype(BF)
        assert xT.shape[0] == F2
        invd_own = np.ones(WP, dtype=np.float32)
        invd_own[:R] = invd[c * R:(c + 1) * R]

        data.append(dict(
            idxA=np.tile(idxa.reshape(-1, 16).T, (8, 1)).copy(),
            idxB=np.tile(idxb.reshape(-1, 16).T, (8, 1)).copy(),
            slots=slots.reshape(BTOT, cfg.WIN).T.astype(BF).copy(),
            invd=invd_own.reshape(W, cfg.WIN).T.copy(),
            memb=memb.reshape(W, cfg.WIN, G).transpose(1, 0, 2).reshape(
                cfg.WIN, W * G).astype(BF).copy(),
            membT=memb.T.astype(BF).copy(),
            xT=xT,
        ))

    x_pad = np.zeros((N, F2), dtype=BF)
    x_pad[:, :cfg.F] = x.astype(BF)

    inv_szt = np.tile(inv_sz, (cfg.WIN, CT)).astype(np.float32)  # [WIN, CT*G]

    struct = dict(
        F2=F2, O2=O2, CT=CT, R=R, W=W, WP=WP, S0=S0,
        nblkA=[int(v) for v in nblkA], nblkB=[int(v) for v in nblkB],
        LA=LA, LB=LB, BTOT=BTOT,
    )
    shared = dict(
        x_pad=x_pad,
        inv_szt=inv_szt,
        iota=np.tile(np.arange(128, dtype=np.float32).astype(BF), (128, 1)).copy(),
    )
    return struct, shared, data


def _prep_weights(cfg, st, inp):
    """Host-side packing of the (replicated) weight/norm tensors (bf16)."""
    H, O, G = cfg.H, cfg.O, cfg.G
    F2, O2, CT = st["F2"], st["O2"], st["CT"]

    def ktiled(wT, fo):  # [H, fo] -> [128, CT*fo] (k-tile major SBUF layout)
        return wT.reshape(CT, 128, fo).transpose(1, 0, 2).reshape(128, CT * fo)

    out = {}
    w1lT = np.zeros((F2, H), np.float32)
    w1lT[:cfg.F] = np.asarray(inp["W1l"], np.float32).T
    w1rT = np.zeros((F2, H), np.float32)
    w1rT[:cfg.F] = np.asarray(inp["W1r"], np.float32).T
    out["w1l"] = w1lT.astype(BF)
    out["w1r"] = w1rT.astype(BF)
    out["b1"] = np.asarray(inp["b1"], np.float32).reshape(1, H).astype(BF)
    for l in (2, 3, 4):
        out[f"w{l}l"] = ktiled(np.asarray(inp[f"W{l}l"], np.float32).T, H).astype(BF)
        out[f"w{l}r"] = ktiled(np.asarray(inp[f"W{l}r"], np.float32).T, H).astype(BF)
        out[f"b{l}"] = np.asarray(inp[f"b{l}"], np.float32).reshape(1, H).astype(BF)
    w5lT = np.zeros((H, O2), np.float32)
    w5lT[:, :O] = np.asarray(inp["W5l"], np.float32).T
    out["w5l"] = ktiled(w5lT, O2).astype(BF)
    out["w5r"] = ktiled(np.asarray(inp["W5r"], np.float32).T, O).astype(BF)
    out["b5"] = np.asarray(inp["b5"], np.float32).reshape(1, O).astype(BF)

    for l in (1, 2, 3, 4):
        a = np.asarray(inp[f"a{l}"], np.float32)
        g = np.asarray(inp[f"g{l}"], np.float32)
        bn = np.asarray(inp[f"bn{l}"], np.float32)
        acoef = 2.0 * a - a * a
        # [128, 4*CT]; col p*CT+ct; params p: 0 alpha, 1 acoef, 2 w, 3 bn
        m = np.zeros((128, 4 * CT), np.float32)
        for ct in range(CT):
            cs = slice(ct * 128, (ct + 1) * 128)
            m[:, 0 * CT + ct] = a[cs]
            m[:, 1 * CT + ct] = acoef[cs]
            m[:, 2 * CT + ct] = g[cs]
            m[:, 3 * CT + ct] = bn[cs]
        out[f"nrm{l}"] = m
    return out


# --------------------------------------------------------------------------
# Device program
# --------------------------------------------------------------------------

def _build(cfg, st):
    N, H, O, G, NC = cfg.N, cfg.H, cfg.O, cfg.G, cfg.NC
    F2, O2, CT = st["F2"], st["O2"], st["CT"]
    R, W, WP, S0 = st["R"], st["W"], st["WP"], st["S0"]
    nblkA, nblkB = st["nblkA"], st["nblkB"]
    LA, LB, BTOT = st["LA"], st["LB"], st["BTOT"]
    CTG = CT * G
    RG = [list(range(NC))]
    TDT = {"fp8e4": FP8, "fp8e3": mybir.dt.float8e3, "bf16": BF16,
           "f32": F32}[cfg.GDT]
    T5DT = BF16
    ABL = os.environ.get("KABL", "none")
    gmaxA = max(nblkA) if nblkA else 1
    gmaxB = max(nblkB) if nblkB else 1
    GMAXBLK = int(os.environ.get("KGMAX", "32"))

    nc = bacc.Bacc(
        "TRN2",
        target_bir_lowering=False,
        debug=False,
        num_devices=NC,
        enable_asserts=False,
        dynamic_dma_scratch_size=65536,
    )

    # ---- I/O ----
    din = {}
    def inp(name, shape, dt):
        din[name] = nc.dram_tensor(name, shape, dt, kind="ExternalInput")
        return din[name]

    x_pad = inp("x_pad", [N, F2], BF16)
    xT = inp("xT", [F2, WP], BF16)
    idxA = inp("idxA", [128, max(LA, 16) // 16], I16)
    idxB = inp("idxB", [128, max(LB, 16) // 16], I16)
    slots = inp("slots", [128, BTOT], BF16)
    invd = inp("invd", [128, W], F32)
    memb = inp("memb", [128, W * G], BF16)
    membT = inp("membT", [G, WP], BF16)
    inv_szt = inp("inv_szt", [128, CTG], F32)
    iota = inp("iota", [128, 128], BF16)
    for l in (1, 2, 3, 4):
        inp(f"nrm{l}", [128, 4 * CT], F32)
    inp("w1l", [F2, H], BF16); inp("w1r", [F2, H], BF16); inp("b1", [1, H], BF16)
    for l in (2, 3, 4):
        inp(f"w{l}l", [128, CT * H], BF16)
        inp(f"w{l}r", [128, CT * H], BF16)
        inp(f"b{l}", [1, H], BF16)
    inp("w5l", [128, CT * O2], BF16); inp("w5r", [128, CT * O], BF16)
    inp("b5", [1, O], BF16)

    out_d = nc.dram_tensor("out", [R, O], F32, kind="ExternalOutput")

    import contextlib
    _ctx = contextlib.ExitStack()
    with tile.TileContext(nc) as tc:
        cpool = _ctx.enter_context(tc.tile_pool(name="cpool", bufs=1))
        xpool = _ctx.enter_context(tc.tile_pool(name="xpool", bufs=1))
        wpool = _ctx.enter_context(tc.tile_pool(name="wpool", bufs=2))
        gpool = _ctx.enter_context(tc.tile_pool(name="gpool", bufs=int(os.environ.get("KGBUFS", "2"))))
        ipool = _ctx.enter_context(tc.tile_pool(name="ipool", bufs=2))
        work = _ctx.enter_context(tc.tile_pool(name="work", bufs=2))
        pseg = _ctx.enter_context(tc.tile_pool(name="pseg", bufs=2, space="PSUM"))
        pstat = _ctx.enter_context(tc.tile_pool(name="pstat", bufs=1, space="PSUM"))
        phwp = _ctx.enter_context(tc.tile_pool(name="phwp", bufs=2, space="PSUM"))
        pwork = _ctx.enter_context(tc.tile_pool(name="pwork", bufs=3, space="PSUM"))
        dram = _ctx.enter_context(tc.tile_pool(name="dram", bufs=1, space="DRAM"))

        # ---- internal DRAM ----
        h_pre = dram.tile([WP, H], BF16, tag="h_pre", name="h_pre")
        t_in = {l: dram.tile([R, H], TDT, tag=f"t_in{l}", name=f"t_in{l}")
                for l in (2, 3, 4)}
        t_in[5] = dram.tile([R, O2], T5DT, tag="t_in5", name="t_in5")
        shared_as = "Shared" if NC > 4 else "Local"
        t_full = {l: dram.tile([N, H], TDT, tag=f"t_full{l}",
                               name=f"t_full{l}", addr_space=shared_as)
                  for l in (2, 3, 4)}
        t_full[5] = dram.tile([N, O2], T5DT, tag="t_full5", name="t_full5",
                              addr_space=shared_as)
        ar_in = {l: dram.tile([128, 2 * CTG], F32, tag=f"ar_in{l}", name=f"ar_in{l}")
                 for l in (1, 2, 3, 4)}
        ar_out = {l: dram.tile([128, 2 * CTG], F32, tag=f"ar_out{l}",
                               name=f"ar_out{l}", addr_space=shared_as)
                  for l in (1, 2, 3, 4)}

        # ---- resident constants ----
        def cload(name):
            t = din[name]
            tl = cpool.tile(list(t.shape), t.dtype, name=f"{name}_sb")
            nc.sync.dma_start(tl[:], t.ap())
            return tl

        idxA_sb = cload("idxA")
        idxB_sb = cload("idxB")
        slots_sb = cload("slots")
        invd_sb = cload("invd")
        memb_sb = cload("memb")
        inv_szt_sb = cload("inv_szt")
        iota_sb = cload("iota")
        nrm_sb = {l: cload(f"nrm{l}") for l in (1, 2, 3, 4)}
        w1l_sb = cload("w1l"); w1r_sb = cload("w1r"); b1_sb = cload("b1")

        identb = cpool.tile([128, 128], BF16, name="identb")
        from concourse.masks import make_identity
        make_identity(nc, identb[:])
        ones1 = cpool.tile([1, 128], BF16, name="ones1")
        nc.vector.memset(ones1[:], 1.0)
        eps_col = cpool.tile([128, 1], F32, name="eps_col")
        nc.vector.memset(eps_col[:], cfg.EPS)

        # SBUF-resident transposed activations: xts_all[:, w, kt, :] is
        # (xn window w, k-tile kt)^T — written by phase_c(l), read by
        # phase_a(l+1) for the hwr term and the T matmuls.
        xts_all = xpool.tile([128, W, CT, 128], BF16, name="xts_all")

        # block offset bookkeeping (same for every layer)
        eA0 = np.concatenate([[0], np.cumsum(nblkA)]).astype(int)   # in blocks
        eB0 = np.concatenate([[0], np.cumsum(nblkB)]).astype(int)
        sc0 = np.concatenate([[0], np.cumsum(np.asarray(nblkA) + np.asarray(nblkB))]).astype(int)

        def load_wr(lw):
            """Load Wr (+bias) for layer lw into SBUF (bf16)."""
            WW = H if lw < 5 else O
            wr = wpool.tile([128, CT * WW], BF16, tag="wr", name=f"wr{lw}")
            nc.sync.dma_start(wr[:], din[f"w{lw}r"].ap())
            br = wpool.tile([1, WW], BF16, tag="br", name=f"br{lw}")
            nc.sync.dma_start(br[:], din[f"b{lw}"].ap())
            return wr, br

        def phase_a(l, wr_br):
            """gather + segment matmul + local term; h_pre/stats (l<=4) or out (l==5)."""
            if l == 1:
                src, elem, sdt = x_pad, F2, BF16
            elif l < 5:
                src, elem, sdt = t_full[l], H, TDT
            else:
                src, elem, sdt = t_full[5], O2, T5DT
            segw = elem if l == 1 else (H if l < 5 else O2)
            WW = H if l < 5 else O
            if l > 1:
                wr, br = wr_br

            if l <= 4:
                stats = pstat.tile([128, 2 * CTG], F32, tag="stats", name=f"stats{l}")

            nbmax = max(nblkA[w] + nblkB[w] for w in range(W))
            for w in range(W):
                bA, bB = nblkA[w], nblkB[w]
                nb = bA + bB
                r0 = w * 128

                # local term: hwr = xn_w @ Wr + b (uses resident xts_all);
                # independent of the gathers/collective -> fills PE idle time.
                if l > 1:
                    phw = phwp.tile([128, WW], F32, tag="hw", name=f"phw_{l}_{w}")
                    for kt in range(CT):
                        nc.tensor.matmul(out=phw[:], lhsT=xts_all[:, w, kt, :],
                                         rhs=wr[:, kt * WW:(kt + 1) * WW],
                                         start=(kt == 0), stop=False)
                    nc.tensor.matmul(out=phw[:], lhsT=ones1[:], rhs=br[:],
                                     start=False, stop=True)

                gA = gB = None
                if bA:
                    gA = gpool.tile([128, gmaxA, elem], sdt, tag="gA", name=f"gA_{l}_{w}")
                    for o in [] if ABL == "nogather" else range(0, bA, GMAXBLK):
                        n = min(GMAXBLK, bA - o)
                        nc.gpsimd.dma_gather(
                            out_ap=gA[:, o:o + n, :],
                            in_ap=src[:S0, :] if S0 < N else src[:, :],
                            idxs_ap=idxA_sb[:, (eA0[w] + o) * 8: (eA0[w] + o + n) * 8],
                            num_idxs=n * 128,
                            num_idxs_reg=n * 128,
                            elem_size=elem,
                        )
                if bB:
                    gB = gpool.tile([128, gmaxB, elem], sdt, tag="gB", name=f"gB_{l}_{w}")
                    for o in [] if ABL == "nogather" else range(0, bB, GMAXBLK):
                        n = min(GMAXBLK, bB - o)
                        nc.gpsimd.dma_gather(
                            out_ap=gB[:, o:o + n, :],
                            in_ap=src[S0:, :],
                            idxs_ap=idxB_sb[:, (eB0[w] + o) * 8: (eB0[w] + o + n) * 8],
                            num_idxs=n * 128,
                            num_idxs_reg=n * 128,
                            elem_size=elem,
                        )
                seg = pseg.tile([128, segw], F32, tag="seg", name=f"seg_{l}_{w}")
                if nb == 0:
                    nc.vector.memset(seg[:], 0.0)
                else:
                    # whole-window indicator build: ind[e, b, s] = (slot[e, b] == s)
                    ind = ipool.tile([128, nbmax, 128], sdt, tag="ind",
                                     name=f"ind_{l}_{w}")
                    sl = slots_sb[:, sc0[w]: sc0[w] + nb]
                    slots_b = bass.AP(tensor=sl.tensor, offset=sl.offset,
                                      ap=[list(sl.ap[0]), list(sl.ap[1]), [0, 128]])
                    iota_b = bass.AP(tensor=iota_sb.tensor, offset=iota_sb.offset,
                                     ap=[list(iota_sb.ap[0]), [0, nb],
                                         list(iota_sb.ap[1])])
                    nc.vector.tensor_tensor(out=ind[:, :nb, :], in0=iota_b,
                                            in1=slots_b, op=ALU.is_equal)
                bi = 0
                for gt, bcnt in ((gA, bA), (gB, bB)):
                    for b in range(bcnt):
                        if ABL == "noseg" and bi > 0:
                            bi += 1
                            continue
                        nc.tensor.matmul(
                            out=seg[:], lhsT=ind[:, bi, :], rhs=gt[:, b, :],
                            start=(bi == 0), stop=(bi == nb - 1 or ABL == "noseg"))
                        bi += 1

                if l == 1:
                    aggs = work.tile([128, F2], BF16, tag="aggs", name=f"aggs_{w}")
                    nc.vector.tensor_scalar(out=aggs[:], in0=seg[:],
                                            scalar1=invd_sb[:, w:w + 1],
                                            scalar2=None, op0=ALU.mult)
                    tp = pwork.tile([F2, 128], BF16, tag="wk", name=f"atp_{w}")
                    nc.tensor.transpose(tp[:], aggs[:], identb[:])
                    aT = work.tile([F2, 128], BF16, tag="aT", name=f"aT_{w}")
                    nc.vector.tensor_copy(aT[:], tp[:])
                    xtt = work.tile([F2, 128], BF16, tag="xtt", name=f"xtt_{w}")
                    nc.sync.dma_start(xtt[:], xT[:, r0:r0 + 128])
                    hp = pwork.tile([128, H], F32, tag="wk", name=f"hp_{w}")
                    nc.tensor.matmul(out=hp[:], lhsT=aT[:], rhs=w1l_sb[:],
                                     start=True, stop=False)
                    nc.tensor.matmul(out=hp[:], lhsT=xtt[:],
                                     rhs=w1r_sb[:], start=False, stop=False)
                    nc.tensor.matmul(out=hp[:], lhsT=ones1[:], rhs=b1_sb[:],
                                     start=False, stop=True)
                    h2 = work.tile([128, H], BF16, tag="h2", name=f"h2_{l}_{w}")
                    nc.vector.tensor_copy(h2[:], hp[:])
                elif l < 5:
                    h1t = work.tile([128, H], F32, tag="h1t", name=f"h1t_{l}_{w}")
                    nc.vector.tensor_scalar(out=h1t[:], in0=seg[:],
                                            scalar1=invd_sb[:, w:w + 1],
                                            scalar2=None, op0=ALU.mult)
                    h2 = work.tile([128, H], BF16, tag="h2", name=f"h2_{l}_{w}")
                    nc.vector.tensor_tensor(out=h2[:], in0=h1t[:], in1=phw[:],
                                            op=ALU.add)
                else:
                    o1 = work.tile([128, O], F32, tag="o1", name=f"o1_{w}")
                    nc.vector.tensor_scalar(out=o1[:], in0=seg[:, :O],
                                            scalar1=invd_sb[:, w:w + 1],
                                            scalar2=None, op0=ALU.mult)
                    o2 = work.tile([128, O], F32, tag="o2", name=f"o2_{w}")
                    nc.vector.tensor_tensor(out=o2[:], in0=o1[:], in1=phw[:],
                                            op=ALU.add)
                    rows = min(128, R - r0)
                    nc.sync.dma_start(out_d[r0:r0 + rows, :], o2[:rows, :])
                    continue

                # stats + h_pre store (l <= 4)
                hsq = work.tile([128, H], BF16, tag="hsq", name=f"hsq_{l}_{w}")
                nc.vector.tensor_tensor(out=hsq[:], in0=h2[:], in1=h2[:],
                                        op=ALU.mult)
                for ct in range(CT):
                    nc.tensor.matmul(
                        out=stats[:, ct * G:(ct + 1) * G],
                        lhsT=h2[:, ct * 128:(ct + 1) * 128],
                        rhs=memb_sb[:, w * G:(w + 1) * G],
                        start=(w == 0 and ct == 0),
                        stop=False)
                    nc.tensor.matmul(
                        out=stats[:, CTG + ct * G: CTG + (ct + 1) * G],
                        lhsT=hsq[:, ct * 128:(ct + 1) * 128],
                        rhs=memb_sb[:, w * G:(w + 1) * G],
                        start=False,
                        stop=(w == W - 1 and ct == CT - 1))
                nc.sync.dma_start(h_pre[r0:r0 + 128, :], h2[:])
            if l <= 4:
                return stats
            return None

        def norm_coeffs(l, stats):
            """AllReduce raw moments; produce A_rhs/B_rhs [G, H] bf16."""
            stsb = work.tile([128, 2 * CTG], F32, tag="stsb", name=f"stsb{l}")
            nc.vector.tensor_copy(stsb[:], stats[:])
            nc.sync.dma_start(ar_in[l][:, :], stsb[:])
            if ABL != "nocc":
                nc.gpsimd.collective_compute(
                    "AllReduce", ALU.add, replica_groups=RG,
                    ins=[ar_in[l].opt()], outs=[ar_out[l].opt()])
            stg = work.tile([128, 2 * CTG], F32, tag="stg", name=f"stg{l}")
            nc.sync.dma_start(stg[:], ar_out[l][:, :])

            nrm = nrm_sb[l]
            mean = work.tile([128, CTG], F32, tag="mean", name=f"mean{l}")
            nc.vector.tensor_tensor(out=mean[:], in0=stg[:, :CTG],
                                    in1=inv_szt_sb[:], op=ALU.mult)
            e2 = work.tile([128, CTG], F32, tag="e2", name=f"e2{l}")
            nc.vector.tensor_tensor(out=e2[:], in0=stg[:, CTG:],
                                    in1=inv_szt_sb[:], op=ALU.mult)
            msq = work.tile([128, CTG], F32, tag="msq", name=f"msq{l}")
            nc.vector.tensor_tensor(out=msq[:], in0=mean[:], in1=mean[:],
                                    op=ALU.mult)
            msq2 = work.tile([128, CTG], F32, tag="msq2", name=f"msq2{l}")
            for ct in range(CT):
                sl = slice(ct * G, (ct + 1) * G)
                nc.vector.tensor_scalar(out=msq2[:, sl], in0=msq[:, sl],
                                        scalar1=nrm[:, 1 * CT + ct: 1 * CT + ct + 1],
                                        scalar2=None, op0=ALU.mult)
            var = work.tile([128, CTG], F32, tag="var", name=f"var{l}")
            nc.vector.tensor_tensor(out=var[:], in0=e2[:], in1=msq2[:],
                                    op=ALU.subtract)
            sd = work.tile([128, CTG], F32, tag="sd", name=f"sd{l}")
            nc.scalar.activation(out=sd[:], in_=var[:], func=AF.Sqrt,
                                 bias=eps_col[:])
            rstd = work.tile([128, CTG], F32, tag="rstd", name=f"rstd{l}")
            nc.vector.reciprocal(rstd[:], sd[:])
            At = work.tile([128, CTG], F32, tag="At", name=f"At{l}")
            for ct in range(CT):
                sl = slice(ct * G, (ct + 1) * G)
                nc.vector.tensor_scalar(out=At[:, sl], in0=rstd[:, sl],
                                        scalar1=nrm[:, 2 * CT + ct: 2 * CT + ct + 1],
                                        scalar2=None, op0=ALU.mult)
            mA = work.tile([128, CTG], F32, tag="mA", name=f"mA{l}")
            nc.vector.tensor_tensor(out=mA[:], in0=mean[:], in1=At[:],
                                    op=ALU.mult)
            Bt = work.tile([128, CTG], F32, tag="Bt", name=f"Bt{l}")
            for ct in range(CT):
                sl = slice(ct * G, (ct + 1) * G)
                nc.vector.tensor_scalar(out=Bt[:, sl], in0=mA[:, sl],
                                        scalar1=nrm[:, 0 * CT + ct: 0 * CT + ct + 1],
                                        scalar2=nrm[:, 3 * CT + ct: 3 * CT + ct + 1],
                                        op0=ALU.mult, op1=ALU.subtract)
            Atb = work.tile([128, CTG], BF16, tag="Atb", name=f"Atb{l}")
            nc.vector.tensor_copy(Atb[:], At[:])
            Btb = work.tile([128, CTG], BF16, tag="Btb", name=f"Btb{l}")
            nc.vector.tensor_copy(Btb[:], Bt[:])
            A_rhs = work.tile([G, H], BF16, tag="A_rhs", name=f"A_rhs{l}")
            B_rhs = work.tile([G, H], BF16, tag="B_rhs", name=f"B_rhs{l}")
            for ct in range(CT):
                tpa = pwork.tile([G, 128], BF16, tag="wk", name=f"tpa{l}_{ct}")
                nc.tensor.transpose(tpa[:], Atb[:, ct * G:(ct + 1) * G], identb[:])
                nc.vector.tensor_copy(A_rhs[:, ct * 128:(ct + 1) * 128], tpa[:])
                tpb = pwork.tile([G, 128], BF16, tag="wk", name=f"tpb{l}_{ct}")
                nc.tensor.transpose(tpb[:], Btb[:, ct * G:(ct + 1) * G], identb[:])
                nc.vector.tensor_copy(B_rhs[:, ct * 128:(ct + 1) * 128], tpb[:])
            return A_rhs, B_rhs

        def phase_c(l, A_rhs, B_rhs):
            """normalize+relu h_pre, fill xts_all, emit t_in(l+1); AllGather."""
            lw = l + 1
            TW = H if lw < 5 else O2
            wl = wpool.tile([128, CT * TW], BF16, tag="wl", name=f"wl{lw}")
            nc.sync.dma_start(wl[:], din[f"w{lw}l"].ap())

            for mt in range(W):
                r0 = mt * 128
                hc = work.tile([128, H], BF16, tag="hc", name=f"hc{l}_{mt}")
                nc.sync.dma_start(hc[:], h_pre[r0:r0 + 128, :])
                mTt = work.tile([G, 128], BF16, tag="mTt", name=f"mTt{l}_{mt}")
                nc.sync.dma_start(mTt[:], membT[:, r0:r0 + 128])
                pa = pwork.tile([128, H], F32, tag="wk", name=f"pa{l}_{mt}")
                nc.tensor.matmul(out=pa[:], lhsT=mTt[:],
                                 rhs=A_rhs[:], start=True, stop=True)
                pb = pwork.tile([128, H], F32, tag="wk", name=f"pb{l}_{mt}")
                nc.tensor.matmul(out=pb[:], lhsT=mTt[:],
                                 rhs=B_rhs[:], start=True, stop=True)
                t0 = work.tile([128, H], F32, tag="t0", name=f"t0{l}_{mt}")
                nc.vector.tensor_tensor(out=t0[:], in0=hc[:], in1=pa[:],
                                        op=ALU.mult)
                t1 = work.tile([128, H], F32, tag="t1", name=f"t1{l}_{mt}")
                nc.vector.tensor_tensor(out=t1[:], in0=t0[:], in1=pb[:],
                                        op=ALU.subtract)
                xn = work.tile([128, H], BF16, tag="xn", name=f"xn{l}_{mt}")
                nc.scalar.activation(out=xn[:], in_=t1[:], func=AF.Relu)
                for kt in range(CT):
                    tp = pwork.tile([128, 128], BF16, tag="wk", name=f"tp{l}_{mt}_{kt}")
                    nc.tensor.transpose(tp[:], xn[:, kt * 128:(kt + 1) * 128], identb[:])
                    nc.vector.tensor_copy(xts_all[:, mt, kt, :], tp[:])
                pt = pwork.tile([128, TW], F32, tag="wk", name=f"pt{l}_{mt}")
                for kt in range(CT):
                    nc.tensor.matmul(out=pt[:], lhsT=xts_all[:, mt, kt, :],
                                     rhs=wl[:, kt * TW:(kt + 1) * TW],
                                     start=(kt == 0), stop=(kt == CT - 1))
                tsb = work.tile([128, TW], TDT if lw < 5 else T5DT, tag="tsb",
                                name=f"tsb{l}_{mt}")
                nc.vector.tensor_copy(tsb[:], pt[:])
                rows = min(128, R - r0)
                nc.sync.dma_start(t_in[lw][r0:r0 + rows, :], tsb[:rows, :])

            if ABL != "nocc":
                nc.gpsimd.collective_compute(
                    "AllGather", ALU.bypass, replica_groups=RG,
                    ins=[t_in[lw].opt()], outs=[t_full[lw].opt()])

        for l in (1, 2, 3, 4):
            wr_br = load_wr(l) if l > 1 else None
            stats = phase_a(l, wr_br)
            A_rhs, B_rhs = norm_coeffs(l, stats)
            phase_c(l, A_rhs, B_rhs)
        phase_a(5, load_wr(5))

        _ctx.close()

    nc.compile()
    return nc


# --------------------------------------------------------------------------
# Entry point
# --------------------------------------------------------------------------

def _run(cfg, inputs, trace=False):
    st, shared, data = _plan(cfg, inputs["x"], inputs["edge_index"], inputs["batch"])
    wts = _prep_weights(cfg, st, inputs)
    nc = _build(cfg, st)
    in_maps = []
    for c in range(cfg.NC):
        m = dict(shared)
        m.update(wts)
        m.update(data[c])
        in_maps.append(m)
    res = bass_utils.run_bass_kernel_spmd(
        nc, in_maps, core_ids=list(range(cfg.NC)), trace=trace)
    out = np.concatenate([res.results[c]["out"] for c in range(cfg.NC)], axis=0)
    return out.astype(np.float32), res


def kernel(**inputs) -> np.ndarray:
    out, _ = _run(CFG, inputs)
    return out
